# revision 1
# baseline (speedup 1.0000x reference)
"""Trainium2 Bass kernel for DPAttention (attention block + residual + LayerNorm).

Sharding: 8 cores = DP2 (batch) x TP4 (head groups of 3 heads).
Core c: b = c//4, g = c%4 -> heads [3g, 3g+3), output rows [512g, 512g+512) of batch b.

Per-core dataflow:
  X^T (f32, SBUF)
    -> Q^T/K^T [d, s] via matmul(lhsT=Wq_slice, rhs=X^T)   (heads 0,1 stacked on 128
       partitions; head 2 in its own 64-partition tile)
    -> V [s, d] via matmul(lhsT=X^T tile, rhs=Wv_slice), stored bf16 with a ones
       column appended per head (gives the softmax denominator for free)
  scores^T [k, q] = matmul(lhsT=K^T tile, rhs=Q^T chunk)   (heads 0/1 run in different
       PE row-groups concurrently)
  e = exp(scores^T * 1/8 + mask_bias_k)  on ScalarE, psum->sbuf bf16
  ctx^T [d+1, q] = sum_k matmul(lhsT=[V|1], rhs=e) ; + rank-1 (1e18 * u) matmul that
       overwrites invalid-query columns with the uniform-attention value u = mean_k V
  normalize by denominator row, write ctx^T bf16
  4-core AllGather of ctx^T -> full [768, 2048]; dynamic-offset DMA reads this core's
       512 query columns -> out dense (bf16) + residual + LayerNorm -> out [512, 768].
"""
import numpy as np
import ml_dtypes

import concourse.bass as bass
import concourse.mybir as mybir
import concourse.tile as tile
from concourse import bacc
from concourse.bass_utils import run_bass_kernel_spmd

F32 = mybir.dt.float32
BF16 = mybir.dt.bfloat16
U32 = mybir.dt.uint32
AF = mybir.ActivationFunctionType
ALU = mybir.AluOpType
AX = mybir.AxisListType

B, S, H, NH, HD = 2, 2048, 768, 12, 64
P = 128
KT = H // P            # 6 contraction tiles over hidden
ST = S // P            # 16 tiles over sequence
TP = 4                 # head groups (tensor-parallel within a batch)
HG = NH // TP          # 3 heads per core
HGD = HG * HD          # 192
SQ = S // TP           # 512 output rows per core
EPS = 1e-5
SCALE = 1.0 / np.sqrt(HD)
NCORES = 8
GROUPS = [[0, 1, 2, 3], [4, 5, 6, 7]]
BIGNEG = -1.0e9
BIGPOS = 1.0e18

_cache = {}

import os
STAGE = int(os.environ.get("KSTAGE", "6"))


def build():
    nc = bacc.Bacc(num_devices=NCORES)

    xt_d = nc.dram_tensor("xt", [H, S], BF16, kind="ExternalInput")
    xres_d = nc.dram_tensor("xres", [SQ, H], F32, kind="ExternalInput")
    wq_d = nc.dram_tensor("wq", [H, HGD], BF16, kind="ExternalInput")
    wk_d = nc.dram_tensor("wk", [H, HGD], BF16, kind="ExternalInput")
    wv_d = nc.dram_tensor("wv", [H, HGD], BF16, kind="ExternalInput")
    bq_d = nc.dram_tensor("bq", [HGD], F32, kind="ExternalInput")
    bk_d = nc.dram_tensor("bk", [HGD], F32, kind="ExternalInput")
    bvr_d = nc.dram_tensor("bvr", [P, HGD], F32, kind="ExternalInput")
    wo_d = nc.dram_tensor("wo", [H, H], BF16, kind="ExternalInput")
    mkb_d = nc.dram_tensor("mkb", [S], F32, kind="ExternalInput")
    gq_d = nc.dram_tensor("gq", [1, S], BF16, kind="ExternalInput")
    lng_d = nc.dram_tensor("lng", [P, H], F32, kind="ExternalInput")
    lnb_d = nc.dram_tensor("lnb", [P, H], F32, kind="ExternalInput")
    xsum_d = nc.dram_tensor("xsum", [SQ], F32, kind="ExternalInput")
    qoff_d = nc.dram_tensor("qoff", [1, 2], U32, kind="ExternalInput")
    out_d = nc.dram_tensor("out", [SQ, H], F32, kind="ExternalOutput")

    with tile.TileContext(nc) as tc:
        with (
            tc.tile_pool(name="wts", bufs=1) as wts,
            tc.tile_pool(name="qkv", bufs=1) as qkv,
            tc.tile_pool(name="dram", bufs=1, space="DRAM") as dram,
        ):
            # ---- load weights / small tensors ----
            wq_sb = wts.tile([P, KT, HGD], BF16)
            wk_sb = wts.tile([P, KT, HGD], BF16)
            wv_sb = wts.tile([P, KT, HGD], BF16)
            nc.sync.dma_start(wq_sb[:], wq_d.rearrange("(kt p) d -> p kt d", p=P))
            nc.sync.dma_start(wk_sb[:], wk_d.rearrange("(kt p) d -> p kt d", p=P))
            nc.sync.dma_start(wv_sb[:], wv_d.rearrange("(kt p) d -> p kt d", p=P))
            wo_sb = wts.tile([P, KT, H], BF16)
            nc.sync.dma_start(wo_sb[:], wo_d.rearrange("(kt p) n -> p kt n", p=P))

            bq_sb = wts.tile([P, 2], F32)
            bk_sb = wts.tile([P, 2], F32)
            nc.gpsimd.dma_start(bq_sb[:, 0:1], bq_d[0:P].rearrange("(p o) -> p o", o=1))
            nc.gpsimd.dma_start(bq_sb[0:HGD - P, 1:2], bq_d[P:HGD].rearrange("(p o) -> p o", o=1))
            nc.gpsimd.dma_start(bk_sb[:, 0:1], bk_d[0:P].rearrange("(p o) -> p o", o=1))
            nc.gpsimd.dma_start(bk_sb[0:HGD - P, 1:2], bk_d[P:HGD].rearrange("(p o) -> p o", o=1))
            bvr_sb = wts.tile([P, HG, HD], F32)
            nc.sync.dma_start(bvr_sb[:], bvr_d.rearrange("p (h d) -> p h d", d=HD))
            mkb_sb = wts.tile([P, ST], F32)
            nc.gpsimd.dma_start(mkb_sb[:], mkb_d.rearrange("(kt p) -> p kt", p=P))
            gq_sb = wts.tile([1, S], BF16)
            nc.gpsimd.dma_start(gq_sb[:], gq_d[:])
            lng_sb = wts.tile([P, H], F32)
            lnb_sb = wts.tile([P, H], F32)
            nc.sync.dma_start(lng_sb[:], lng_d[:])
            nc.sync.dma_start(lnb_sb[:], lnb_d[:])
            xres_sb = wts.tile([P, SQ // P, H], F32)
            nc.sync.dma_start(xres_sb[:], xres_d.rearrange("(t p) n -> p t n", p=P))
            qoff_sb = wts.tile([1, 2], U32)
            nc.gpsimd.dma_start(qoff_sb[:], qoff_d[:])
            xsum_sb = wts.tile([P, SQ // P], F32)
            nc.gpsimd.dma_start(xsum_sb[:], xsum_d.rearrange("(t p) -> p t", p=P))

            ones_sb = wts.tile([P, 1], BF16)
            nc.gpsimd.memset(ones_sb[:], 1.0)

            # ---- persistent intermediate tiles ----
            qt_sb = qkv.tile([P, S], BF16)      # Q^T heads 0,1 (rows 0:64 / 64:128)
            qt2_sb = qkv.tile([P, S], BF16)     # Q^T head 2 (rows 64:128 zero)
            # K^T zero-padded to full 128 contraction rows per head (keeps PE full)
            ktz_sb = qkv.tile([P, HG, S], BF16)
            v_sb = qkv.tile([P, ST, HG, P], BF16)   # V + ones col + zero pad per head
            u_sb = qkv.tile([1, HG, P], BF16)       # mean_k V (+1 slot), zero padded
            nc.gpsimd.memset(ktz_sb[:], 0.0)
            nc.gpsimd.memset(qt2_sb[HD:P, :], 0.0)
            nc.gpsimd.memset(v_sb[:], 0.0)
            nc.gpsimd.memset(u_sb[:], 0.0)
            ctxa_sb = qkv.tile([P, S], BF16)   # ctx^T heads 0,1
            ctxb_sb = qkv.tile([HD, S], BF16)  # ctx^T head 2

            nc.gpsimd.memset(v_sb[:, :, :, HD:HD + 1], 1.0)

            # ================= projections (V overlapped into attention) ========
            with tc.tile_pool(name="xt", bufs=1) as xtp:
                xt_sb = xtp.tile([P, KT, S], BF16)
                xt_r = xt_d.rearrange("(kt p) s -> p kt s", p=P)
                for kt in range(KT):
                    nc.sync.dma_start(xt_sb[:, kt, :], xt_r[:, kt, :])

                # Q^T / K^T: two M-passes (128 for heads 0,1; 64 for head 2)
                pps_ctx = __import__("contextlib").ExitStack()
                pps = pps_ctx.enter_context(tc.tile_pool(name="pps", bufs=2, space="PSUM"))
                for w_sb, b_sb, is_k in (
                    (wq_sb, bq_sb, False),
                    (wk_sb, bk_sb, True),
                ):
                    for mp, (m0, msz) in enumerate(((0, P), (P, HGD - P))):
                        for qc in range(S // 512):
                            qs = slice(qc * 512, (qc + 1) * 512)
                            ps = pps.tile([P, 512], F32, tag="proj")
                            for kt in range(KT):
                                nc.tensor.matmul(
                                    ps[:msz],
                                    w_sb[:, kt, m0:m0 + msz],
                                    xt_sb[:, kt, qc * 512:(qc + 1) * 512],
                                    start=(kt == 0), stop=(kt == KT - 1),
                                )
                            if not is_k:
                                d_sb = qt_sb if mp == 0 else qt2_sb
                                nc.vector.tensor_scalar_add(
                                    d_sb[:msz, qs], ps[:msz], b_sb[:msz, mp:mp + 1])
                            elif mp == 0:
                                nc.vector.tensor_scalar_add(
                                    ktz_sb[0:HD, 0, qs], ps[0:HD], b_sb[0:HD, 0:1])
                                nc.vector.tensor_scalar_add(
                                    ktz_sb[HD:P, 1, qs], ps[HD:P], b_sb[HD:P, 0:1])
                            else:
                                nc.vector.tensor_scalar_add(
                                    ktz_sb[0:HD, 2, qs], ps[0:HD], b_sb[0:HD, 1:2])

                pps_ctx.close()

                def emit_vproj(st):
                    ps = cps.tile([P, HGD], F32, tag="c", name=f"vp{st}")
                    for kt in range(KT):
                        nc.tensor.matmul(
                            ps[:], xt_sb[:, kt, st * P:(st + 1) * P], wv_sb[:, kt, :],
                            start=(kt == 0), stop=(kt == KT - 1),
                        )
                    nc.vector.tensor_tensor(
                        v_sb[:, st, :, 0:HD], ps[:].rearrange("p (h d) -> p h d", d=HD),
                        bvr_sb[:], op=ALU.add,
                    )

                def emit_u():
                    ups = cps.tile([1, HGD], F32, tag="c", name="ups")
                    for st in range(ST):
                        nc.tensor.matmul(
                            ups[:], ones_sb[:], v_sb[:, st, :, 0:HD],
                            start=(st == 0), stop=(st == ST - 1),
                        )
                    nc.vector.tensor_scalar_mul(
                        u_sb[0:1, :, 0:HD],
                        ups[:].rearrange("p (h d) -> p h d", d=HD), 1.0 / S)
                    nc.gpsimd.memset(u_sb[:, :, HD:HD + 1], 1.0)

                # ============= attention (software-pipelined units) =============
                QH = S // 1024  # 2 query halves per head
                units = [(h, qh) for qh in range(QH) for h in range(HG)] if STAGE >= 2 else []

                from contextlib import ExitStack
                attn_ctx = ExitStack()
                epool = attn_ctx.enter_context(tc.tile_pool(name="epool", bufs=2))
                sps = attn_ctx.enter_context(tc.tile_pool(name="sps", bufs=2, space="PSUM"))
                cps = attn_ctx.enter_context(tc.tile_pool(name="cps", bufs=2, space="PSUM"))
                npool = attn_ctx.enter_context(tc.tile_pool(name="npool", bufs=2))

                e_tiles = {}
                c_tiles = {}

                def emit_scores_kt(i, kt):
                    h, qh = units[i]
                    e_t = e_tiles[i]
                    ps = sps.tile([P, 1024], F32, tag="sc", name=f"sc{i}_{kt}")
                    lhsT = ktz_sb[:, h, kt * P:(kt + 1) * P]
                    for sub in range(2):
                        q0 = qh * 1024 + sub * 512
                        rhs = (qt_sb[:, q0:q0 + 512] if h < 2
                               else qt2_sb[:, q0:q0 + 512])
                        nc.tensor.matmul(ps[:, sub * 512:(sub + 1) * 512], lhsT, rhs,
                                         start=True, stop=True)
                    nc.scalar.activation(e_t[:, kt, :], ps[:], AF.Exp,
                                         bias=mkb_sb[:, kt:kt + 1], scale=float(SCALE))

                def emit_ctx_kt(i, kt):
                    h, qh = units[i]
                    e_t = e_tiles[i]
                    pc = c_tiles[i]
                    for sub in range(2):
                        nc.tensor.matmul(
                            pc[:, sub * 512:(sub + 1) * 512], v_sb[:, kt, h, :],
                            e_t[:, kt, sub * 512:(sub + 1) * 512],
                            start=(kt == 0), stop=False,
                        )

                def emit_ctx_tail(i):
                    h, qh = units[i]
                    pc = c_tiles[i]
                    rden = dram.tile([1, 1024], F32, tag="rden", bufs=2,
                                     name=f"rden{i}")
                    den = npool.tile([1, 1024], F32, tag="den")
                    q0 = qh * 1024
                    for sub in range(2):
                        nc.tensor.matmul(pc[:, sub * 512:(sub + 1) * 512], u_sb[0:1, h, :],
                                         gq_sb[0:1, q0 + sub * 512:q0 + (sub + 1) * 512],
                                         start=False, stop=True)
                    nc.vector.tensor_copy(den[:], pc[HD:HD + 1, :])
                    nc.vector.reciprocal_approx_fast(den[:], den[:])
                    nc.sync.dma_start(rden[:], den[:])
                    rb = npool.tile([HD, 1024], F32, tag="rb")
                    nc.sync.dma_start(rb[:], rden[0:1, :].to_broadcast((HD, 1024)))
                    dst = (ctxa_sb[HD * h:HD * (h + 1), q0:q0 + 1024] if h < 2
                           else ctxb_sb[:, q0:q0 + 1024])
                    nc.vector.tensor_tensor(dst, pc[0:HD, :], rb[:], op=ALU.mult)

                ag_in = [dram.tile([HGD, 1024], BF16, name="agi0"),
                         dram.tile([HGD, 1024], BF16, name="agi1")]
                ag_out = dram.tile([QH, TP, HGD, 1024], BF16)

                def emit_ag(qh):
                    q0 = qh * 1024
                    nc.sync.dma_start(ag_in[qh][0:P, :], ctxa_sb[:, q0:q0 + 1024])
                    nc.sync.dma_start(ag_in[qh][P:HGD, :], ctxb_sb[:, q0:q0 + 1024])
                    nc.gpsimd.collective_compute(
                        "AllGather", ALU.bypass, replica_groups=GROUPS,
                        ins=[ag_in[qh].opt()], outs=[ag_out[qh].opt()],
                    )

                for i in range(len(units) + 1):
                    if i < len(units):
                        e_tiles[i] = epool.tile([P, ST, 1024], BF16, tag="e", name=f"e{i}")
                    if i > 0:
                        c_tiles[i - 1] = cps.tile([P, 1024], F32, tag="c", name=f"c{i-1}")
                    for kt in range(ST):
                        if i < len(units):
                            emit_scores_kt(i, kt)
                        if i == 0:
                            emit_vproj(kt)
                        if i > 0:
                            emit_ctx_kt(i - 1, kt)
                    if i == 0:
                        emit_u()
                    if i > 0:
                        emit_ctx_tail(i - 1)
                        del e_tiles[i - 1]
                        if STAGE >= 3 and (i - 1) % HG == HG - 1:
                            emit_ag((i - 1) // HG)
                attn_ctx.close()

            # ================= gather ctx across the TP group =================
            if STAGE >= 3:
                with tc.tile_critical():
                    with nc.gpsimd.register("qx") as qx_reg:
                        nc.gpsimd.reg_load(qx_reg, qoff_sb[0:1, 0:1])
                        qx_v = nc.gpsimd.snap(qx_reg)
                    with nc.gpsimd.register("qi") as qi_reg:
                        nc.gpsimd.reg_load(qi_reg, qoff_sb[0:1, 1:2])
                        qi_v = nc.gpsimd.snap(qi_reg)

                ctxg_sb = qkv.tile([P, KT, SQ], BF16)
                nc.gpsimd.dma_start(
                    ctxg_sb[:],
                    ag_out.rearrange("x g d q -> x (g d) q")
                    .rearrange("x (kt p) q -> p kt x q", p=P)[
                        :, :, bass.ds(qx_v, 1), bass.ds(qi_v, SQ)],
                )

            # ================= out dense + residual + LayerNorm =================
            if STAGE < 4:
                with tc.tile_pool(name="dummy", bufs=1) as dpool:
                    for st4 in range(SQ // P):
                        d_sb = dpool.tile([P, H], F32, tag="d")
                        nc.vector.tensor_copy(d_sb[:], xres_sb[:, st4, :])
                        nc.sync.dma_start(out_d[st4 * P:(st4 + 1) * P, :], d_sb[:])
            NST = SQ // P
            with tc.tile_pool(name="ops", bufs=2, space="PSUM") as ops, \
                 tc.tile_pool(name="lnp", bufs=1) as lnp:
                h_all = lnp.tile([P, NST, H], F32)
                mu_all = lnp.tile([P, NST], F32)
                var_all = lnp.tile([P, NST], F32)
                sq_tmp = lnp.tile([P, H], F32, bufs=2)
                for st4 in range(NST if STAGE >= 4 else 0):
                    ps = ops.tile([P, H], F32, tag="od", name=f"od{st4}")
                    for kt in range(KT):
                        lhsT = ctxg_sb[:, kt, st4 * P:(st4 + 1) * P]
                        nc.tensor.matmul(ps[:, 0:512], lhsT, wo_sb[:, kt, 0:512],
                                         start=(kt == 0), stop=(kt == KT - 1))
                        nc.tensor.matmul(ps[:, 512:H], lhsT, wo_sb[:, kt, 512:H],
                                         start=(kt == 0), stop=(kt == KT - 1))
                    # h = out_dense + (x + bo); mean via ACT copy-accumulate
                    psc = lnp.tile([P, H], F32, tag="psc", bufs=2, name=f"psc{st4}")
                    nc.scalar.activation(psc[:], ps[:], AF.Identity,
                                         accum_out=mu_all[:, st4:st4 + 1])
                    nc.vector.tensor_tensor(h_all[:, st4, :], psc[:],
                                            xres_sb[:, st4, :], op=ALU.add)

                if STAGE >= 4 and STAGE < 5:
                    for st4 in range(NST):
                        nc.sync.dma_start(out_d[st4 * P:(st4 + 1) * P, :],
                                          h_all[:, st4, :])
                elif STAGE >= 5:
                    # mu = (psum_rowsum + xres_rowsum)/H ; var via ACT Square-accum
                    nc.vector.tensor_tensor(mu_all[:], mu_all[:], xsum_sb[:], op=ALU.add)
                    nc.vector.tensor_scalar_mul(mu_all[:], mu_all[:], 1.0 / H)
                    negmu = lnp.tile([P, NST], F32)
                    nc.vector.tensor_scalar_mul(negmu[:], mu_all[:], -1.0)
                    for st4 in range(NST):
                        nc.scalar.activation(sq_tmp[:], h_all[:, st4, :], AF.Square,
                                             bias=negmu[:, st4:st4 + 1],
                                             accum_out=var_all[:, st4:st4 + 1])
                    nc.vector.tensor_scalar_mul(var_all[:], var_all[:], 1.0 / H)
                    nc.vector.tensor_scalar_add(var_all[:], var_all[:], EPS)
                    # rstd = 1/sqrt(var) with one Newton step (batched over stiles)
                    std0 = lnp.tile([P, NST], F32)
                    nc.scalar.activation(std0[:], var_all[:], AF.Sqrt)
                    y0 = lnp.tile([P, NST], F32)
                    nc.vector.reciprocal(y0[:], std0[:])
                    t0 = lnp.tile([P, NST], F32)
                    nc.vector.tensor_tensor(t0[:], y0[:], y0[:], op=ALU.mult)
                    nc.vector.tensor_tensor(t0[:], t0[:], var_all[:], op=ALU.mult)
                    nc.vector.tensor_scalar_mul(t0[:], t0[:], -0.5)
                    nc.vector.tensor_scalar_add(t0[:], t0[:], 1.5)
                    rstd = lnp.tile([P, NST], F32)
                    nc.vector.tensor_tensor(rstd[:], y0[:], t0[:], op=ALU.mult)

                    for st4 in range(NST):
                        hc = lnp.tile([P, H], F32, tag="hc", bufs=2, name=f"hc{st4}")
                        nc.vector.tensor_scalar_sub(hc[:], h_all[:, st4, :],
                                                    mu_all[:, st4:st4 + 1])
                        o_sb = lnp.tile([P, H], F32, tag="o", bufs=2, name=f"o{st4}")
                        nc.vector.scalar_tensor_tensor(
                            out=o_sb[:], in0=hc[:], scalar=rstd[:, st4:st4 + 1],
                            in1=lng_sb[:], op0=ALU.mult, op1=ALU.mult)
                        nc.vector.tensor_tensor(o_sb[:], o_sb[:], lnb_sb[:], op=ALU.add)
                        nc.sync.dma_start(out_d[st4 * P:(st4 + 1) * P, :], o_sb[:])

    nc.compile()
    return nc


def _prep_inputs(inputs):
    hs = np.asarray(inputs["hidden_states"], dtype=np.float32)
    am = np.asarray(inputs["attention_mask"], dtype=np.float32)
    Wq = np.asarray(inputs["Wq"], dtype=np.float32)
    Wk = np.asarray(inputs["Wk"], dtype=np.float32)
    Wv = np.asarray(inputs["Wv"], dtype=np.float32)
    Wo = np.asarray(inputs["Wo"], dtype=np.float32)
    bq = np.asarray(inputs["bq"], dtype=np.float32)
    bk = np.asarray(inputs["bk"], dtype=np.float32)
    bv = np.asarray(inputs["bv"], dtype=np.float32)
    bo = np.asarray(inputs["bo"], dtype=np.float32)
    lng = np.asarray(inputs["ln_gamma"], dtype=np.float32)
    lnb = np.asarray(inputs["ln_beta"], dtype=np.float32)

    wo_bf = Wo.astype(ml_dtypes.bfloat16)
    lng_rep = np.ascontiguousarray(np.broadcast_to(lng, (P, H)))
    lnb_rep = np.ascontiguousarray(np.broadcast_to(lnb, (P, H)))

    in_maps = []
    for c in range(NCORES):
        b, g = c // TP, c % TP
        cs = slice(HGD * g, HGD * (g + 1))
        mk = np.where(am[b] >= 0, 0.0, BIGNEG).astype(np.float32)
        gqv = np.where(am[b] >= 0, 0.0, BIGPOS).astype(ml_dtypes.bfloat16)[None, :]
        in_maps.append({
            "xt": np.ascontiguousarray(hs[b].T).astype(ml_dtypes.bfloat16),
            "xres": np.ascontiguousarray(hs[b, SQ * g:SQ * (g + 1)] + bo),
            "wq": np.ascontiguousarray(Wq[:, cs]).astype(ml_dtypes.bfloat16),
            "wk": np.ascontiguousarray(Wk[:, cs]).astype(ml_dtypes.bfloat16),
            "wv": np.ascontiguousarray(Wv[:, cs]).astype(ml_dtypes.bfloat16),
            "bq": np.ascontiguousarray(bq[cs]),
            "bk": np.ascontiguousarray(bk[cs]),
            "bvr": np.ascontiguousarray(np.broadcast_to(bv[cs], (P, HGD))),
            "wo": np.ascontiguousarray(wo_bf),
            "mkb": mk,
            "gq": np.ascontiguousarray(gqv),
            "lng": lng_rep,
            "lnb": lnb_rep,
            "xsum": np.ascontiguousarray(
                (hs[b, SQ * g:SQ * (g + 1)] + bo).sum(axis=1).astype(np.float32)),
            "qoff": np.array([[g // 2, (g % 2) * SQ]], dtype=np.uint32),
        })
    return in_maps


def _run(inputs, trace=False, trace_cores=None):
    if "nc" not in _cache:
        _cache["nc"] = build()
    nc = _cache["nc"]
    in_maps = _prep_inputs(inputs)
    res = run_bass_kernel_spmd(
        nc, in_maps, list(range(NCORES)), trace=trace,
        trace_cores=trace_cores,
    )
    out = np.empty((B, S, H), dtype=np.float32)
    for c in range(NCORES):
        b, g = c // TP, c % TP
        out[b, SQ * g:SQ * (g + 1)] = res.results[c]["out"]
    return out, res


def kernel(**inputs) -> np.ndarray:
    out, _ = _run(inputs)
    return out



# revision 6
# speedup vs baseline: 1.2821x; 1.2821x over previous
"""Trainium2 Bass kernel for DPAttention (attention block + residual + LayerNorm).

Sharding: 8 cores = DP2 (batch) x TP4 (head groups of 3 heads).
Core c: b = c//4, g = c%4 -> heads [3g, 3g+3), output rows [512g, 512g+512) of batch b.

Mask-compaction: the attention mask is a kernel input; invalid keys (mask<0)
contribute exp(-1e9)=0 to softmax, and invalid queries all produce the same
uniform-attention value u = mean_k V (over ALL keys). Host compacts:
  - keys: valid positions only, padded to NKP (mult of 128); pad rows get
    bias -1e9 so e=0.
  - queries: valid positions per 512-row output block, each block padded to
    NQB; 4 blocks -> compact query axis of 4*NQB per batch.
Per-core work drops ~4x on exp/scores/ctx. Invalid output rows use
u = xbar@Wv + bv (xbar = host-computed column mean of X), and the final
out-dense/LN runs on [valid-block | invalid-block] compact rows; the host
scatters rows back.

Per-core dataflow:
  K^T/Q^T [192, *] via matmul(lhsT=W_slice, rhs=X^T-compact), ACT adds bias
  V [NKP, 192] + ones column (softmax denominator for free)
  scores^T [k,q] per (head, block-pair): 64-contraction matmuls, no padding
  e = exp(scores/8 + bias_k) on ScalarE -> bf16
  ctx^T [65, q] accumulated over k-blocks; den row = ctx row 64; normalize
  2 AllGathers (one per block pair) of [192, 2*NQB+1] (last col = u)
  dynamic-offset gather of this core's NQB columns + u -> out dense (valid
  rows) + uo broadcast (invalid rows) + residual + LayerNorm -> [NROW, 768].
"""
import numpy as np
import ml_dtypes

import concourse.bass as bass
import concourse.mybir as mybir
import concourse.tile as tile
from concourse import bacc
from concourse.bass_utils import run_bass_kernel_spmd

F32 = mybir.dt.float32
BF16 = mybir.dt.bfloat16
U32 = mybir.dt.uint32
AF = mybir.ActivationFunctionType
ALU = mybir.AluOpType

B, S, H, NH, HD = 2, 2048, 768, 12, 64
P = 128
KT = H // P            # 6 contraction tiles over hidden
TP = 4                 # head groups (tensor-parallel within a batch)
HG = NH // TP          # 3 heads per core
HGD = HG * HD          # 192
SQ = S // TP           # 512 output rows per core
EPS = 1e-5
SCALE = 1.0 / np.sqrt(HD)
NCORES = 8
GROUPS = [[0, 1, 2, 3], [4, 5, 6, 7]]
BIGNEG = -1.0e9

_cache = {}


def build(NKP, NQB, NROW):
    KB = NKP // P          # key blocks
    NQW = 2 * NQB          # unit width (block pair)
    NQA = 4 * NQB          # full compact query axis
    AGW = NQW + 1          # AllGather payload width (+u column)
    NST = NROW // P        # LN row tiles
    # out-dense m-blocks covering rows [0, NQB): sizes
    vmbs = []
    r = 0
    while r < NQB:
        vmbs.append((r, min(P, NQB - r)))
        r += P

    nc = bacc.Bacc(num_devices=NCORES)

    xkt_d = nc.dram_tensor("xkt", [H, NKP], BF16, kind="ExternalInput")
    xqt_d = nc.dram_tensor("xqt", [H, NQA], BF16, kind="ExternalInput")
    xbart_d = nc.dram_tensor("xbart", [P, KT], BF16, kind="ExternalInput")
    xres_d = nc.dram_tensor("xres", [NROW, H], F32, kind="ExternalInput")
    wq_d = nc.dram_tensor("wq", [H, HGD], BF16, kind="ExternalInput")
    wk_d = nc.dram_tensor("wk", [H, HGD], BF16, kind="ExternalInput")
    wv_d = nc.dram_tensor("wv", [H, HGD], BF16, kind="ExternalInput")
    bq_d = nc.dram_tensor("bq", [HGD], F32, kind="ExternalInput")
    bk_d = nc.dram_tensor("bk", [HGD], F32, kind="ExternalInput")
    bvr_d = nc.dram_tensor("bvr", [P, HGD], F32, kind="ExternalInput")
    bvu_d = nc.dram_tensor("bvu", [HGD], F32, kind="ExternalInput")
    wo_d = nc.dram_tensor("wo", [H, H], BF16, kind="ExternalInput")
    mkb_d = nc.dram_tensor("mkb", [NKP], F32, kind="ExternalInput")
    lng_d = nc.dram_tensor("lng", [P, H], F32, kind="ExternalInput")
    lnb_d = nc.dram_tensor("lnb", [P, H], F32, kind="ExternalInput")
    xsum_d = nc.dram_tensor("xsum", [NROW], F32, kind="ExternalInput")
    qoff_d = nc.dram_tensor("qoff", [1, 2], U32, kind="ExternalInput")
    out_d = nc.dram_tensor("out", [NROW, H], F32, kind="ExternalOutput")

    with tile.TileContext(nc) as tc:
        with (
            tc.tile_pool(name="wts", bufs=1) as wts,
            tc.tile_pool(name="qkv", bufs=1) as qkv,
            tc.tile_pool(name="dram", bufs=1, space="DRAM") as dram,
        ):
            # ---- load weights / small tensors ----
            wq_sb = wts.tile([P, KT, HGD], BF16)
            wk_sb = wts.tile([P, KT, HGD], BF16)
            wv_sb = wts.tile([P, KT, HGD], BF16)
            nc.sync.dma_start(wq_sb[:], wq_d.rearrange("(kt p) d -> p kt d", p=P))
            nc.sync.dma_start(wk_sb[:], wk_d.rearrange("(kt p) d -> p kt d", p=P))
            nc.sync.dma_start(wv_sb[:], wv_d.rearrange("(kt p) d -> p kt d", p=P))
            wo_sb = wts.tile([P, KT, H], BF16)
            nc.sync.dma_start(wo_sb[:], wo_d.rearrange("(kt p) n -> p kt n", p=P))

            bq_sb = wts.tile([P, 2], F32)
            bk_sb = wts.tile([P, 2], F32)
            bvu_sb = wts.tile([P, 2], F32)
            for b_sb, b_d in ((bq_sb, bq_d), (bk_sb, bk_d), (bvu_sb, bvu_d)):
                nc.gpsimd.dma_start(b_sb[:, 0:1], b_d[0:P].rearrange("(p o) -> p o", o=1))
                nc.gpsimd.dma_start(b_sb[0:HGD - P, 1:2],
                                    b_d[P:HGD].rearrange("(p o) -> p o", o=1))
            bvr_sb = wts.tile([P, HG, HD], F32)
            nc.sync.dma_start(bvr_sb[:], bvr_d.rearrange("p (h d) -> p h d", d=HD))
            mkb_sb = wts.tile([P, KB], F32)
            nc.gpsimd.dma_start(mkb_sb[:], mkb_d.rearrange("(kt p) -> p kt", p=P))
            xbart_sb = wts.tile([P, KT], BF16)
            nc.sync.dma_start(xbart_sb[:], xbart_d[:])
            lng_sb = wts.tile([P, H], F32)
            lnb_sb = wts.tile([P, H], F32)
            nc.sync.dma_start(lng_sb[:], lng_d[:])
            nc.sync.dma_start(lnb_sb[:], lnb_d[:])
            xres_sb = wts.tile([P, NST, H], F32)
            nc.sync.dma_start(xres_sb[:], xres_d.rearrange("(t p) n -> p t n", p=P))
            qoff_sb = wts.tile([1, 2], U32)
            nc.gpsimd.dma_start(qoff_sb[:], qoff_d[:])
            xsum_sb = wts.tile([P, NST], F32)
            nc.gpsimd.dma_start(xsum_sb[:], xsum_d.rearrange("(t p) -> p t", p=P))

            # ---- persistent intermediate tiles ----
            kta_sb = qkv.tile([P, NKP], BF16)    # K^T heads 0,1
            ktb_sb = qkv.tile([HD, NKP], BF16)   # K^T head 2
            qta_sb = qkv.tile([P, NQA], BF16)    # Q^T heads 0,1
            qtb_sb = qkv.tile([HD, NQA], BF16)   # Q^T head 2
            v_sb = qkv.tile([P, KB, HG, HD + 1], BF16)  # V + ones col per head
            u_sb = qkv.tile([P, 2], F32)         # u^T = xbar@Wv + bv (192 in 2 cols)
            ctxa_sb = qkv.tile([P, 2, AGW], BF16)   # ctx^T heads 0,1 per block pair
            ctxb_sb = qkv.tile([HD, 2, AGW], BF16)  # ctx^T head 2

            nc.gpsimd.memset(v_sb[:, :, :, HD:HD + 1], 1.0)

            # ================= K/Q projections =====================
            with tc.tile_pool(name="xt", bufs=1) as xtp:
                xkt_sb = xtp.tile([P, KT, NKP], BF16)
                xqt_sb = xtp.tile([P, KT, NQA], BF16)
                xk_r = xkt_d.rearrange("(kt p) s -> p kt s", p=P)
                xq_r = xqt_d.rearrange("(kt p) s -> p kt s", p=P)
                for kt in range(KT):
                    nc.sync.dma_start(xkt_sb[:, kt, :], xk_r[:, kt, :])
                for kt in range(KT):
                    nc.sync.dma_start(xqt_sb[:, kt, :], xq_r[:, kt, :])

                pps_ctx = __import__("contextlib").ExitStack()
                pps = pps_ctx.enter_context(tc.tile_pool(name="pps", bufs=2, space="PSUM"))

                def emit_proj(x_sb, w_sb, b_sb, da, db, width):
                    nchunk = (width + 511) // 512
                    for mp, (m0, msz) in enumerate(((0, P), (P, HGD - P))):
                        for qc in range(nchunk):
                            q0 = qc * 512
                            qsz = min(512, width - q0)
                            ps = pps.tile([P, 512], F32, tag="proj")
                            for kt in range(KT):
                                nc.tensor.matmul(
                                    ps[:msz, 0:qsz],
                                    w_sb[:, kt, m0:m0 + msz],
                                    x_sb[:, kt, q0:q0 + qsz],
                                    start=(kt == 0), stop=(kt == KT - 1),
                                )
                            dst = (da[:, q0:q0 + qsz] if mp == 0
                                   else db[:, q0:q0 + qsz])
                            nc.scalar.activation(dst, ps[:msz, 0:qsz], AF.Identity,
                                                 bias=b_sb[:msz, mp:mp + 1])

                emit_proj(xkt_sb, wk_sb, bk_sb, kta_sb, ktb_sb, NKP)
                emit_proj(xqt_sb, wq_sb, bq_sb, qta_sb, qtb_sb, NQA)
                pps_ctx.close()

                # ============= attention (software-pipelined units) =============
                units = [(h, bp) for bp in range(2) for h in range(HG)]

                from contextlib import ExitStack
                attn_ctx = ExitStack()
                epool = attn_ctx.enter_context(tc.tile_pool(name="epool", bufs=2))
                sps = attn_ctx.enter_context(tc.tile_pool(name="sps", bufs=2, space="PSUM"))
                cps = attn_ctx.enter_context(tc.tile_pool(name="cps", bufs=2, space="PSUM"))
                npool = attn_ctx.enter_context(tc.tile_pool(name="npool", bufs=2))

                e_tiles = {}
                c_tiles = {}

                def kt_of(h):
                    return (kta_sb[HD * h:HD * (h + 1), :] if h < 2
                            else ktb_sb[:, :])

                def qt_of(h):
                    return (qta_sb[HD * h:HD * (h + 1), :] if h < 2
                            else qtb_sb[:, :])

                def emit_scores_kb(i, kb):
                    h, bp = units[i]
                    e_t = e_tiles[i]
                    ps = sps.tile([P, 2, 512], F32, tag="sc", name=f"sc{i}_{kb}")
                    lhsT = kt_of(h)[:, kb * P:(kb + 1) * P]
                    for sub in range(2):
                        q0 = bp * NQW + sub * NQB
                        nc.tensor.matmul(ps[:, sub, 0:NQB], lhsT,
                                         qt_of(h)[:, q0:q0 + NQB],
                                         start=True, stop=True)
                    nc.scalar.activation(
                        e_t[:, kb, :].rearrange("p (s q) -> p s q", q=NQB),
                        ps[:, :, 0:NQB], AF.Exp,
                        bias=mkb_sb[:, kb:kb + 1], scale=float(SCALE))

                def emit_vproj(kb):
                    ps = cps.tile([P, HGD], F32, tag="c", name=f"vp{kb}")
                    for kt in range(KT):
                        nc.tensor.matmul(
                            ps[:], xkt_sb[:, kt, kb * P:(kb + 1) * P], wv_sb[:, kt, :],
                            start=(kt == 0), stop=(kt == KT - 1),
                        )
                    nc.vector.tensor_tensor(
                        v_sb[:, kb, :, 0:HD], ps[:].rearrange("p (h d) -> p h d", d=HD),
                        bvr_sb[:], op=ALU.add,
                    )

                def emit_u():
                    for mp, (m0, msz) in enumerate(((0, P), (P, HGD - P))):
                        ups = cps.tile([P, 1], F32, tag="c", name=f"ups{mp}")
                        for kt in range(KT):
                            nc.tensor.matmul(
                                ups[0:msz, :], wv_sb[:, kt, m0:m0 + msz],
                                xbart_sb[:, kt:kt + 1],
                                start=(kt == 0), stop=(kt == KT - 1),
                            )
                        nc.vector.tensor_scalar_add(
                            u_sb[0:msz, mp:mp + 1], ups[0:msz, :],
                            bvu_sb[0:msz, mp:mp + 1])
                    # write u into the AG payload's last column (both block pairs)
                    for bp in range(2):
                        nc.vector.tensor_copy(ctxa_sb[:, bp, NQW:NQW + 1],
                                              u_sb[:, 0:1])
                        nc.vector.tensor_copy(ctxb_sb[:, bp, NQW:NQW + 1],
                                              u_sb[0:HD, 1:2])

                def emit_ctx_kb(i, kb):
                    h, bp = units[i]
                    e_t = e_tiles[i]
                    pc = c_tiles[i]
                    for sub in range(2):
                        nc.tensor.matmul(
                            pc[0:HD + 1, sub, 0:NQB], v_sb[:, kb, h, :],
                            e_t[:, kb, sub * NQB:(sub + 1) * NQB],
                            start=(kb == 0), stop=(kb == KB - 1),
                        )

                def emit_ctx_tail(i):
                    h, bp = units[i]
                    pc = c_tiles[i]
                    rden = dram.tile([1, NQW], F32, tag="rden", bufs=2,
                                     name=f"rden{i}")
                    den = npool.tile([1, NQW], F32, tag="den")
                    nc.vector.tensor_copy(
                        den[:].rearrange("p (s q) -> p s q", q=NQB),
                        pc[HD:HD + 1, :, 0:NQB])
                    nc.vector.reciprocal_approx_fast(den[:], den[:])
                    nc.sync.dma_start(rden[:], den[:])
                    rb = npool.tile([HD, NQW], F32, tag="rb")
                    nc.sync.dma_start(rb[:], rden[0:1, :].to_broadcast((HD, NQW)))
                    dst = (ctxa_sb[HD * h:HD * (h + 1), bp, 0:NQW] if h < 2
                           else ctxb_sb[:, bp, 0:NQW])
                    nc.vector.tensor_tensor(
                        dst.rearrange("p (s q) -> p s q", q=NQB),
                        pc[0:HD, :, 0:NQB],
                        rb[:].rearrange("p (s q) -> p s q", q=NQB), op=ALU.mult)

                ag_in = [dram.tile([HGD, AGW], BF16, name="agi0"),
                         dram.tile([HGD, AGW], BF16, name="agi1")]
                ag_out = dram.tile([2, TP, HGD, AGW], BF16)

                def emit_ag(bp):
                    nc.sync.dma_start(ag_in[bp][0:P, :], ctxa_sb[:, bp, :])
                    nc.sync.dma_start(ag_in[bp][P:HGD, :], ctxb_sb[:, bp, :])
                    nc.gpsimd.collective_compute(
                        "AllGather", ALU.bypass, replica_groups=GROUPS,
                        ins=[ag_in[bp].opt()], outs=[ag_out[bp].opt()],
                    )

                for i in range(len(units) + 1):
                    if i < len(units):
                        e_tiles[i] = epool.tile([P, KB, NQW], BF16, tag="e",
                                                name=f"e{i}")
                    if i > 0:
                        c_tiles[i - 1] = cps.tile([P, 2, 512], F32, tag="c",
                                                  name=f"c{i-1}")
                    for kb in range(KB):
                        if i < len(units):
                            emit_scores_kb(i, kb)
                        if i == 0 and kb < KB:
                            emit_vproj(kb)
                        if i > 0:
                            emit_ctx_kb(i - 1, kb)
                    if i == 0:
                        emit_u()
                    if i > 0:
                        emit_ctx_tail(i - 1)
                        del e_tiles[i - 1]
                        if (i - 1) % HG == HG - 1:
                            emit_ag((i - 1) // HG)
                attn_ctx.close()

            # ================= gather ctx for this core's block =================
            with tc.tile_critical():
                with nc.gpsimd.register("qx") as qx_reg:
                    nc.gpsimd.reg_load(qx_reg, qoff_sb[0:1, 0:1])
                    qx_v = nc.gpsimd.snap(qx_reg)
                with nc.gpsimd.register("qi") as qi_reg:
                    nc.gpsimd.reg_load(qi_reg, qoff_sb[0:1, 1:2])
                    qi_v = nc.gpsimd.snap(qi_reg)

            ag_r = (ag_out.rearrange("x g d q -> x (g d) q")
                    .rearrange("x (kt p) q -> p kt x q", p=P))
            ctxg_sb = qkv.tile([P, KT, NQB], BF16)
            nc.gpsimd.dma_start(
                ctxg_sb[:], ag_r[:, :, bass.ds(qx_v, 1), bass.ds(qi_v, NQB)])
            ctxu_sb = qkv.tile([P, KT, 1], BF16)
            nc.gpsimd.dma_start(
                ctxu_sb[:], ag_r[:, :, bass.ds(qx_v, 1), NQW:NQW + 1])

            # ================= out dense + residual + LayerNorm =================
            with tc.tile_pool(name="ops", bufs=2, space="PSUM") as ops, \
                 tc.tile_pool(name="lnp", bufs=1) as lnp:
                # uo = u @ Wo  (for invalid-query rows)
                uops = ops.tile([1, H], F32, tag="uo")
                for kt in range(KT):
                    nc.tensor.matmul(uops[:, 0:512], ctxu_sb[:, kt, :],
                                     wo_sb[:, kt, 0:512],
                                     start=(kt == 0), stop=(kt == KT - 1))
                    nc.tensor.matmul(uops[:, 512:H], ctxu_sb[:, kt, :],
                                     wo_sb[:, kt, 512:H],
                                     start=(kt == 0), stop=(kt == KT - 1))
                uo_sb = lnp.tile([1, H], F32)
                nc.vector.tensor_copy(uo_sb[:], uops[:])
                uod = dram.tile([1, H], F32, name="uod")
                nc.sync.dma_start(uod[:], uo_sb[:])
                uoB = lnp.tile([P, H], F32)
                nc.sync.dma_start(uoB[:], uod[0:1, :].to_broadcast((P, H)))

                h_all = lnp.tile([P, NST, H], F32)
                mu_all = lnp.tile([P, NST], F32)
                var_all = lnp.tile([P, NST], F32)
                sq_tmp = lnp.tile([P, H], F32, bufs=2)
                for st in range(NST):
                    r0 = st * P
                    vsz = max(0, min(P, NQB - r0))   # valid-dense rows in this tile
                    if vsz > 0:
                        ps = ops.tile([P, H], F32, tag="od", name=f"od{st}")
                        for kt in range(KT):
                            lhsT = ctxg_sb[:, kt, r0:r0 + vsz]
                            nc.tensor.matmul(ps[0:vsz, 0:512], lhsT,
                                             wo_sb[:, kt, 0:512],
                                             start=(kt == 0), stop=(kt == KT - 1))
                            nc.tensor.matmul(ps[0:vsz, 512:H], lhsT,
                                             wo_sb[:, kt, 512:H],
                                             start=(kt == 0), stop=(kt == KT - 1))
                        psc = lnp.tile([P, H], F32, tag="psc", bufs=2,
                                       name=f"psc{st}")
                        nc.scalar.activation(psc[0:vsz], ps[0:vsz], AF.Identity,
                                             accum_out=mu_all[0:vsz, st:st + 1])
                        nc.vector.tensor_tensor(h_all[0:vsz, st, :], psc[0:vsz],
                                                xres_sb[0:vsz, st, :], op=ALU.add)
                    # invalid-query rows: h = xres + uo (partition-aligned chunks)
                    s = vsz
                    while s < P:
                        m = 128 if s == 0 else (64 if s % 64 == 0 else 32)
                        e = min(s + m, P)
                        nc.vector.scalar_tensor_tensor(
                            out=h_all[s:e, st, :], in0=xres_sb[s:e, st, :],
                            scalar=1.0, in1=uoB[s:e, :],
                            op0=ALU.mult, op1=ALU.add,
                            accum_out=mu_all[s:e, st:st + 1])
                        s = e

                # mu = (accum + xsum)/H ; var via ACT Square-accum
                nc.vector.tensor_tensor(mu_all[:], mu_all[:], xsum_sb[:], op=ALU.add)
                nc.vector.tensor_scalar_mul(mu_all[:], mu_all[:], 1.0 / H)
                negmu = lnp.tile([P, NST], F32)
                nc.vector.tensor_scalar_mul(negmu[:], mu_all[:], -1.0)
                for st in range(NST):
                    nc.scalar.activation(sq_tmp[:], h_all[:, st, :], AF.Square,
                                         bias=negmu[:, st:st + 1],
                                         accum_out=var_all[:, st:st + 1])
                nc.vector.tensor_scalar_mul(var_all[:], var_all[:], 1.0 / H)
                nc.vector.tensor_scalar_add(var_all[:], var_all[:], EPS)
                # rstd = 1/sqrt(var) with one Newton step
                std0 = lnp.tile([P, NST], F32)
                nc.scalar.activation(std0[:], var_all[:], AF.Sqrt)
                y0 = lnp.tile([P, NST], F32)
                nc.vector.reciprocal(y0[:], std0[:])
                t0 = lnp.tile([P, NST], F32)
                nc.vector.tensor_tensor(t0[:], y0[:], y0[:], op=ALU.mult)
                nc.vector.tensor_tensor(t0[:], t0[:], var_all[:], op=ALU.mult)
                nc.vector.tensor_scalar_mul(t0[:], t0[:], -0.5)
                nc.vector.tensor_scalar_add(t0[:], t0[:], 1.5)
                rstd = lnp.tile([P, NST], F32)
                nc.vector.tensor_tensor(rstd[:], y0[:], t0[:], op=ALU.mult)

                for st in range(NST):
                    hc = lnp.tile([P, H], F32, tag="hc", bufs=2, name=f"hc{st}")
                    nc.vector.tensor_scalar_sub(hc[:], h_all[:, st, :],
                                                mu_all[:, st:st + 1])
                    o_sb = lnp.tile([P, H], F32, tag="o", bufs=2, name=f"o{st}")
                    nc.vector.scalar_tensor_tensor(
                        out=o_sb[:], in0=hc[:], scalar=rstd[:, st:st + 1],
                        in1=lng_sb[:], op0=ALU.mult, op1=ALU.mult)
                    nc.vector.tensor_tensor(o_sb[:], o_sb[:], lnb_sb[:], op=ALU.add)
                    nc.sync.dma_start(out_d[st * P:(st + 1) * P, :], o_sb[:])

    nc.compile()
    return nc


def _geometry(am):
    valid = am >= 0
    vidx = [np.where(valid[b])[0] for b in range(B)]
    NKP = int(-(-max(len(v) for v in vidx) // P) * P)
    bidx = {}
    iidx = {}
    for b in range(B):
        for g in range(TP):
            lo, hi = SQ * g, SQ * (g + 1)
            m = (vidx[b] >= lo) & (vidx[b] < hi)
            bidx[(b, g)] = vidx[b][m]
            inv = np.where(~valid[b, lo:hi])[0] + lo
            iidx[(b, g)] = inv
    maxv = max(len(v) for v in bidx.values())
    maxi = max(len(v) for v in iidx.values())
    NQB = int(-(-max(maxv, 1) // 32) * 32)
    need = NQB + int(-(-max(maxi, 1) // 32) * 32)
    NROW = int(-(-need // P) * P)
    return vidx, bidx, iidx, NKP, NQB, NROW


def _prep_inputs(inputs, geom):
    vidx, bidx, iidx, NKP, NQB, NROW = geom
    NQA = 4 * NQB
    hs = np.asarray(inputs["hidden_states"], dtype=np.float32)
    Wq = np.asarray(inputs["Wq"], dtype=np.float32)
    Wk = np.asarray(inputs["Wk"], dtype=np.float32)
    Wv = np.asarray(inputs["Wv"], dtype=np.float32)
    Wo = np.asarray(inputs["Wo"], dtype=np.float32)
    bq = np.asarray(inputs["bq"], dtype=np.float32)
    bk = np.asarray(inputs["bk"], dtype=np.float32)
    bv = np.asarray(inputs["bv"], dtype=np.float32)
    bo = np.asarray(inputs["bo"], dtype=np.float32)
    lng = np.asarray(inputs["ln_gamma"], dtype=np.float32)
    lnb = np.asarray(inputs["ln_beta"], dtype=np.float32)

    wo_bf = Wo.astype(ml_dtypes.bfloat16)
    lng_rep = np.ascontiguousarray(np.broadcast_to(lng, (P, H)))
    lnb_rep = np.ascontiguousarray(np.broadcast_to(lnb, (P, H)))

    # per-batch compacted tensors
    xkt = []
    xqt = []
    xbart = []
    mkb = []
    for b in range(B):
        xk = np.zeros((H, NKP), dtype=ml_dtypes.bfloat16)
        xk[:, :len(vidx[b])] = hs[b].T[:, vidx[b]].astype(ml_dtypes.bfloat16)
        xkt.append(xk)
        xq = np.zeros((H, NQA), dtype=ml_dtypes.bfloat16)
        for g in range(TP):
            bi = bidx[(b, g)]
            xq[:, NQB * g:NQB * g + len(bi)] = \
                hs[b].T[:, bi].astype(ml_dtypes.bfloat16)
        xqt.append(xq)
        xbart.append(np.ascontiguousarray(
            hs[b].mean(axis=0).reshape(KT, P).T.astype(ml_dtypes.bfloat16)))
        mk = np.zeros(NKP, dtype=np.float32)
        mk[len(vidx[b]):] = BIGNEG
        mkb.append(mk)

    in_maps = []
    for c in range(NCORES):
        b, g = c // TP, c % TP
        cs = slice(HGD * g, HGD * (g + 1))
        bi = bidx[(b, g)]
        ii = iidx[(b, g)]
        xres = np.zeros((NROW, H), dtype=np.float32)
        xres[0:len(bi)] = hs[b, bi] + bo
        xres[NQB:NQB + len(ii)] = hs[b, ii] + bo
        xsum = np.zeros(NROW, dtype=np.float32)
        xsum[0:len(bi)] = xres[0:len(bi)].sum(axis=1)
        in_maps.append({
            "xkt": xkt[b],
            "xqt": xqt[b],
            "xbart": xbart[b],
            "xres": xres,
            "wq": np.ascontiguousarray(Wq[:, cs]).astype(ml_dtypes.bfloat16),
            "wk": np.ascontiguousarray(Wk[:, cs]).astype(ml_dtypes.bfloat16),
            "wv": np.ascontiguousarray(Wv[:, cs]).astype(ml_dtypes.bfloat16),
            "bq": np.ascontiguousarray(bq[cs]),
            "bk": np.ascontiguousarray(bk[cs]),
            "bvr": np.ascontiguousarray(np.broadcast_to(bv[cs], (P, HGD))),
            "bvu": np.ascontiguousarray(bv[cs]),
            "wo": np.ascontiguousarray(wo_bf),
            "mkb": mkb[b],
            "lng": lng_rep,
            "lnb": lnb_rep,
            "xsum": xsum,
            "qoff": np.array([[g // 2, (g % 2) * NQB]], dtype=np.uint32),
        })
    return in_maps


def _run(inputs, trace=False, trace_cores=None):
    am = np.asarray(inputs["attention_mask"], dtype=np.float32)
    geom = _geometry(am)
    _, bidx, iidx, NKP, NQB, NROW = geom
    key = (NKP, NQB, NROW)
    if key not in _cache:
        _cache[key] = build(*key)
    nc = _cache[key]
    in_maps = _prep_inputs(inputs, geom)
    res = run_bass_kernel_spmd(
        nc, in_maps, list(range(NCORES)), trace=trace,
        trace_cores=trace_cores,
    )
    out = np.empty((B, S, H), dtype=np.float32)
    for c in range(NCORES):
        b, g = c // TP, c % TP
        r = res.results[c]["out"]
        bi = bidx[(b, g)]
        ii = iidx[(b, g)]
        out[b, bi] = r[0:len(bi)]
        out[b, ii] = r[NQB:NQB + len(ii)]
    return out, res


def kernel(**inputs) -> np.ndarray:
    out, _ = _run(inputs)
    return out
